# revision 10
# baseline (speedup 1.0000x reference)
"""Trainium2 Bass kernel for nn_FACoef — col-tiled chain-matvec version.

Math: with the column-sum chain c_{k+1} = x^T c_k (c_0 = 1),
1^T c_k = 1^T x^k 1, so s_i(b) = 1^T x_b^(i+2) 1 = sum(c_{i+2}(b)):
run 5 chain eras and free-axis-reduce each era's output. No row-sum
vector, no dot products.

Per-era structure (the trick): instead of loading each sample's x as the
matmul STATIONARY (128-col weight load per sample per era), the chain
vector c̃_k(b) is the stationary ([128, c+1], zero-padded so the single
output row lands at psum partition 32q+c) and x_b streams as the MOVING
operand. tile_position=(0, 32q) runs 4 samples concurrently on the four
32-col groups of the PE array, so one era of 32 samples costs ~8 x 128
moving cycles instead of 32 weight loads. The psum rows tiles are
zeroed once at start; each era only rewrites rows 32q..32q+7, so the
other rows stay zero (slot c=7 uses an 8-wide stationary with
start=True to clear/write its strip's 8 rows).

Era output rows (c'_{k+1} as [1,128] rows scattered at partitions
32q+c) are transposed back to columns with ONE selector matmul per era:
lhsT = rows tile (bf16, fast weight load), rhs = 0/1 selector Esel whose
columns pick up each sample's row into a staircase layout with the
leading zeros the next era's zero-padded stationaries need.
s_{k-1}(b) = sum(c_{k+1}(b)) is one VectorE free-axis reduce of the
psum rows tile (fp32), landing on partition 32q+c.

x is cast to bf16 and pre-transposed to [N, BPC, N] on the host: halves
HBM traffic and makes each DMA descriptor an 8 KiB contiguous read.
Total rel err ~2.7e-3 (vs 2e-2 gate), dominated by the bf16 cast of x.

Sharding: pure data parallel, batch dim split across 8 cores.
"""

import numpy as np

B, N = 2048, 128
ROWS, COLS = 4, 4
N_CORES = 8
BPC = B // N_CORES   # samples per core
S = 32               # samples per tile
T = BPC // S         # tiles per core
GW = 36              # spad group width (staircase, X_7 = 35)
NZ = 8               # slots per col-group
SPW = 4 * GW         # spad width (144)
NERA = ROWS + 1      # chain eras (c_1..c_5)

_cache = {}


def _XC(c):
    return c * (c + 3) // 2


def _patch_tail_drain():
    """walrus CoreV3 setupSyncWait rejects instructions carrying several
    semaphore waits; TileContext's kernel-tail drain collects one wait per
    unobserved logical proc. Split them one wait per drain instruction."""
    import concourse.tile as tile
    from concourse import mybir
    from concourse.vector_clock import ScopedClock

    if getattr(tile.TileContext, "_drain_split_patched", False):
        return

    def _drain_and_barrier(self, tick_clock, wait_clock):
        nc = self.nc
        drain_inst = nc.sync.drain()
        wait_clock.add_sem_waits(
            drain_inst.ins, ScopedClock({None: tick_clock.global_clock})
        )
        si = drain_inst.ins.sync_info
        waits = list(si.on_wait) if si is not None and si.on_wait else []
        if len(waits) > 1:
            drain_inst.ins.sync_info = mybir.SyncInfo(
                on_wait=[waits[0]], on_update=list(si.on_update or [])
            )
            for w in waits[1:]:
                extra = nc.sync.drain()
                extra.ins.sync_info = mybir.SyncInfo(on_wait=[w], on_update=[])

        nc.all_engine_barrier()
        assert self.sems is not None
        popped = nc._tile_sem_poison_stack.pop()
        assert popped is self._sem_poison
        nc.clear_and_free_semaphores(list(self.sems.allocated().values()))
        nc.all_engine_barrier()

    tile.TileContext._drain_and_barrier = _drain_and_barrier
    tile.TileContext._drain_split_patched = True


def _split_multi_waits(nc):
    """walrus accepts at most one sync wait per instruction (two for
    EventSemaphore). Hoist extra waits onto same-engine NOPs inserted
    immediately before the carrying instruction."""
    from concourse import mybir

    n_split = 0
    for bb in nc.main_func.blocks:
        new = []
        for inst in bb.instructions:
            si = inst.sync_info
            waits = list(si.on_wait) if si is not None and si.on_wait else []
            cap = 2 if isinstance(inst, mybir.InstEventSemaphore) else 1
            if len(waits) > cap:
                for k, w in enumerate(waits[:-cap]):
                    nop = mybir.InstNoOp(name=f"{inst.name}-wsplit{k}", ins=[], outs=[])
                    nop.engine = inst.engine
                    nop.sync_info = mybir.SyncInfo(on_wait=[w], on_update=[])
                    nc.register_instruction(nop)
                    new.append(nop)
                    n_split += 1
                inst.sync_info = mybir.SyncInfo(
                    on_wait=waits[-cap:], on_update=list(si.on_update or [])
                )
            new.append(inst)
        bb.instructions[:] = new
    return n_split


def _consolidate_pe_incs(nc):
    """Every TensorE matmul carries a +1 semaphore inc (~26 ns issue tail
    each). Consumers only wait at era boundaries, so batch the increments:
    strip per-mm incs and emit one accumulated inc at each waited value."""
    from concourse import mybir
    import bisect

    waited = {}
    for bb in nc.main_func.blocks:
        for ins in bb.instructions:
            si = ins.sync_info
            if si is None:
                continue
            for w in si.on_wait or []:
                waited.setdefault(w.ant_name, set()).add(w.wait_value)

    for bb in nc.main_func.blocks:
        pe_insts = [
            ins
            for ins in bb.instructions
            if isinstance(ins, mybir.InstMatmult)
            and ins.sync_info is not None
            and ins.sync_info.on_update
        ]
        by_sem = {}
        for ins in pe_insts:
            for u in ins.sync_info.on_update:
                if u.update_mode == "sem-inc":
                    by_sem.setdefault(u.ant_name, []).append((ins, u))

        for sem, pairs in by_sem.items():
            wvals = waited.get(sem, set())
            cum = 0
            kept_cums = []
            for idx, (ins, u) in enumerate(pairs):
                cum += u.update_value
                keep = cum in wvals or idx == len(pairs) - 1
                si = ins.sync_info
                others = [
                    x
                    for x in si.on_update
                    if not (x.ant_name == sem and x.update_mode == "sem-inc")
                ]
                if keep:
                    others.append(u)  # original +1 inc (hw requires value 1)
                    kept_cums.append(cum)
                ins.sync_info = mybir.SyncInfo(
                    on_wait=list(si.on_wait or []), on_update=others
                )
            # remap every wait on this sem from raw counts to kept-inc counts
            for bb2 in nc.main_func.blocks:
                for ins in bb2.instructions:
                    si = ins.sync_info
                    if si is None or not si.on_wait:
                        continue
                    changed = False
                    new_waits = []
                    for w in si.on_wait:
                        if w.ant_name == sem and w.wait_mode == "sem-ge-imm":
                            n = bisect.bisect_left(kept_cums, w.wait_value) + 1
                            assert n <= len(kept_cums), (sem, w.wait_value)
                            new_waits.append(
                                mybir.SyncWait(
                                    sync_type="semaphore",
                                    id=w.id,
                                    ant_name=sem,
                                    wait_mode="sem-ge-imm",
                                    wait_value=n,
                                )
                            )
                            changed = True
                        else:
                            new_waits.append(w)
                    if changed:
                        ins.sync_info = mybir.SyncInfo(
                            on_wait=new_waits, on_update=list(si.on_update or [])
                        )


def _build_nc(reps=1):
    import concourse.bass as bass
    import concourse.tile as tile
    from concourse import mybir

    _patch_tail_drain()
    f32 = mybir.dt.float32
    bf16 = mybir.dt.bfloat16
    AX = mybir.AxisListType
    OP = mybir.AluOpType

    nc = bass.Bass()
    x_in = nc.declare_dram_parameter("x", [N, BPC, N], bf16, isOutput=False)
    esel_in = nc.declare_dram_parameter("esel", [N, SPW], bf16, isOutput=False)
    epi_in = nc.declare_dram_parameter("epi", [N, 5 * ROWS * T], f32, isOutput=False)
    y_out = nc.declare_dram_parameter("y", [N, T], f32, isOutput=True)

    IL = 4  # tiles interleaved per round

    with tile.TileContext(nc) as tc:
        with (
            tc.tile_pool(name="xp", bufs=12) as xp,
            # psum banks: rows 4 (bank-padded: PE-writes must not share a
            # bank with concurrent ScalarE/DVE reads) + sel 3 = 7
            tc.tile_pool(name="rowp", bufs=1, space="PSUM") as rowp,
            tc.tile_pool(name="selp", bufs=3, space="PSUM") as selp,
            tc.tile_pool(name="rsp", bufs=IL) as rsp,
            tc.tile_pool(name="spp", bufs=2 * IL) as spp,
            tc.tile_pool(name="constp", bufs=1) as constp,
            tc.tile_pool(name="smallp", bufs=1) as smallp,
        ):
            # constants
            esel = constp.tile([N, SPW], bf16)
            nc.sync.dma_start(esel[:], esel_in[:])
            epi = constp.tile([N, 5 * ROWS * T], f32)
            nc.sync.dma_start(epi[:], epi_in[:])
            # ZOB: cols 0..7 zero, col 8 ones (era-1 stationaries)
            zob = constp.tile([N, 9], bf16)
            nc.vector.memset(zob[:], 0.0)
            nc.vector.memset(zob[:, 8:9], 1.0)
            # persistent, pre-zeroed psum rows tiles (one bank each): eras
            # only rewrite rows 32q..32q+7, the rest stay zero forever
            rows_psum = [
                rowp.tile([N, 512], f32, name=f"rw{i}", tag=f"rw{i}")
                for i in range(IL)
            ]
            for rw in rows_psum:
                nc.vector.memset(rw[:], 0.0)
            # PE warm-up during the first x DMA: ~20 junk matmuls push the
            # HAM activity window so real eras start at 2.4 GHz, not 1.2
            warm = rowp.tile([N, 512], f32, name="warm", tag="warm")
            for _ in range(20):
                nc.tensor.matmul(
                    warm[:, 0:N], esel[:, 0:N], esel[:, 0:N],
                    skip_group_check=True,
                )

            xf = {}
            H = S // 2

            def dma_tile(t):
                if t == 0:
                    # two half-tile DMAs for the first tile: era-1 matmuls
                    # on the first 16 samples start while the second half
                    # is still in flight (ramp)
                    xa = xp.tile([N, H, N], bf16, name="xta", tag="xta")
                    xb = xp.tile([N, H, N], bf16, name="xtb", tag="xtb")
                    nc.sync.dma_start(xa[:, :, :], x_in[:, 0:H, :])
                    nc.sync.dma_start(xb[:, :, :], x_in[:, H:S, :])
                    xf[t] = (xa, xb)
                else:
                    xt = xp.tile([N, S, N], bf16, name="xt", tag="xt")
                    nc.sync.dma_start(
                        xt[:, :, :], x_in[:, t * S : (t + 1) * S, :]
                    )
                    xf[t] = xt

            def chain_mms(t, k, spad_prev):
                """era-k matvecs for tile t: 32 col-tiled MMs, rows out."""
                rows_ps = rows_psum[t % IL]
                for c in range(NZ - 1, -1, -1):
                    for q in range(4):
                        s = 8 * q + c
                        if k == 0:
                            lhsT = zob[:, 1:9] if c == 7 else zob[:, 8 - c : 9]
                        else:
                            xc = _XC(c)
                            lhsT = spad_prev[:, GW * q + xc - c : GW * q + xc + 1]
                        m = NZ if c == 7 else c + 1
                        if isinstance(xf[t], tuple):
                            rhs = xf[t][s // H][:, s % H, :]
                        else:
                            rhs = xf[t][:, s, :]
                        nc.tensor.matmul(
                            rows_ps[32 * q : 32 * q + m, 0:N],
                            lhsT,
                            rhs,
                            start=(c == 7),
                            stop=(c == 0),
                            tile_position=(0, 32 * q),
                            skip_group_check=True,
                        )

            for _rep in range(reps):
                for t in range(IL):
                    dma_tile(t)

                for tq in range(T // IL):
                    quad = tuple(IL * tq + i for i in range(IL))
                    t0 = IL * tq
                    if tq + 1 < T // IL:
                        for t in range(IL * tq + IL, IL * tq + 2 * IL):
                            dma_tile(t)
                    pq = tq & 1
                    sredq = smallp.tile(
                        [N, ROWS * IL], f32, name=f"sred{pq}", tag=f"sred{pq}"
                    )
                    spads = {}
                    for k in range(NERA):
                        rows_sb = {}
                        for t in quad:
                            chain_mms(t, k, spads.get(t))
                            rows_sb[t] = rsp.tile(
                                [N, N], bf16, name="rs", tag="rs"
                            )
                            # alternate copy engine to balance load
                            if k % 2 == 0:
                                nc.scalar.copy(
                                    rows_sb[t][:, :],
                                    rows_psum[t % IL][:, 0:N],
                                )
                            else:
                                nc.vector.tensor_copy(
                                    rows_sb[t][:, :],
                                    rows_psum[t % IL][:, 0:N],
                                )
                        for t in quad:
                            if k >= 1:
                                # s_{k-1}(b) = sum_j c_{k+1}(b)[j]
                                col = (k - 1) * IL + (t - t0)
                                nc.vector.tensor_reduce(
                                    sredq[:, col : col + 1],
                                    rows_sb[t][:, :],
                                    axis=AX.X,
                                    op=OP.add,
                                )
                            if k < NERA - 1:
                                sel_ps = selp.tile(
                                    [N, SPW], f32, name="sel", tag="sel"
                                )
                                nc.tensor.matmul(sel_ps[:, :], rows_sb[t], esel[:])
                                spads[t] = spp.tile(
                                    [N, SPW], bf16, name="sp", tag="sp"
                                )
                                nc.scalar.copy(spads[t][:, :], sel_ps[:, :])

                    # per-quad epilogue: y cols t0..t0+IL-1 DMA out while the
                    # next quad is still streaming (hides the y-DMA latency)
                    KT = ROWS * T
                    KQ = ROWS * IL

                    def qv(block):
                        # [N, (k, quad-t)] view of a k-major [N, KT] epi block
                        return epi[:, block * KT : (block + 1) * KT].rearrange(
                            "p (k t) -> p k t", k=ROWS, t=T
                        )[:, :, t0 : t0 + IL]

                    sq = sredq[:].rearrange("p (k t) -> p k t", k=ROWS, t=IL)
                    sig = smallp.tile([N, KQ], f32, name=f"sg{pq}", tag=f"sg{pq}")
                    sigv = sig[:].rearrange("p (k t) -> p k t", k=ROWS, t=IL)
                    nc.vector.tensor_mul(sigv, sq, qv(0))
                    acc = smallp.tile([N, KQ], f32, name=f"ac{pq}", tag=f"ac{pq}")
                    accv = acc[:].rearrange("p (k t) -> p k t", k=ROWS, t=IL)
                    # acc = W3*sig; acc = (acc + W_j)*sig for j=2,1,0
                    nc.vector.tensor_mul(accv, qv(1), sigv)
                    for idx, j in enumerate((2, 1, 0)):
                        a2 = smallp.tile(
                            [N, KQ], f32, name=f"a{j}{pq}", tag=f"a{j}{pq}"
                        )
                        a2v = a2[:].rearrange("p (k t) -> p k t", k=ROWS, t=IL)
                        nc.vector.tensor_add(a2v, accv, qv(2 + idx))
                        a3 = smallp.tile(
                            [N, KQ], f32, name=f"b{j}{pq}", tag=f"b{j}{pq}"
                        )
                        a3v = a3[:].rearrange("p (k t) -> p k t", k=ROWS, t=IL)
                        nc.vector.tensor_mul(a3v, a2v, sigv)
                        acc, accv = a3, a3v
                    ysb = smallp.tile([N, IL], f32, name=f"y{pq}", tag=f"y{pq}")
                    yin = acc[:].rearrange("p (k t) -> p t k", k=ROWS, t=IL)
                    nc.vector.tensor_reduce(ysb[:], yin, axis=AX.X, op=OP.add)
                    nc.sync.dma_start(y_out[:, t0 : t0 + IL], ysb[:])

    _consolidate_pe_incs(nc)
    _split_multi_waits(nc)
    return nc


def _host_tables(coef):
    n = np.float64(N * N)
    ii = np.arange(ROWS, dtype=np.float64)[:, None]
    jj = np.arange(COLS, dtype=np.float64)[None, :]
    w = (coef.astype(np.float64) * n ** (-(ii + jj + 2.0))).astype(np.float64)
    # out = sum_ij coef_ij s_i^(j+1) / n^(i+j+2) = sum_i sig_i*(w_i0 + sig_i*(
    #   w_i1 + sig_i*(w_i2 + sig_i*w_i3))) with sig_i = s_i * sc_i, absorbing
    # sc_i into w: w'_ij = w_ij / sc_i^(j+1). Use sc_i = n^-((i+3)/4) to keep
    # sig_i O(1) and w' balanced.
    beta = (ii[:, 0] + 3.0) / 4.0
    sc = n ** (-beta)
    wp = w / sc[:, None] ** (jj + 1.0)
    return wp.astype(np.float32), sc.astype(np.float32)


def _host_consts():
    import ml_dtypes

    bf = ml_dtypes.bfloat16
    esel = np.zeros((N, SPW), np.float32)
    for q in range(4):
        for c in range(NZ):
            esel[32 * q + c, GW * q + _XC(c)] = 1.0
    return np.ascontiguousarray(esel.astype(bf))


def _host_epi(coef):
    wp, sc = _host_tables(np.asarray(coef))
    KT = ROWS * T
    epi = np.zeros((N, 5 * KT), np.float32)
    rows = np.array([32 * q + c for q in range(4) for c in range(NZ)])
    for k in range(ROWS):
        epi[rows, k * T : (k + 1) * T] = sc[k]
        for idx, j in enumerate((3, 2, 1, 0)):
            epi[rows, (1 + idx) * KT + k * T : (1 + idx) * KT + (k + 1) * T] = wp[
                k, j
            ]
    return epi


def _host_x(x):
    """Cast the [B, N, N] fp32 input to the device layout/dtype:
    per-core [N, BPC, N] bf16 (partition = row index)."""
    import ml_dtypes

    x = np.asarray(x, dtype=np.float32)
    return [
        np.ascontiguousarray(
            x[c * BPC : (c + 1) * BPC]
            .transpose(1, 0, 2)
            .astype(ml_dtypes.bfloat16)
        )
        for c in range(N_CORES)
    ]


def kernel(x, coef):
    from concourse.bass_utils import run_bass_kernel_spmd

    if "nc" not in _cache:
        _cache["nc"] = _build_nc()
    nc = _cache["nc"]

    esel = _host_consts()
    epi = _host_epi(coef)
    xs = _host_x(x)
    in_maps = []
    for c in range(N_CORES):
        in_maps.append({"x": xs[c], "esel": esel, "epi": epi})
    res = run_bass_kernel_spmd(nc, in_maps, list(range(N_CORES)))
    rows = np.array([32 * q + c for q in range(4) for c in range(NZ)])
    y = np.concatenate(
        [
            np.asarray(res.results[c]["y"])[rows, :].T.reshape(-1)
            for c in range(N_CORES)
        ]
    )
    return y.astype(np.float32)

